# revision 1
# baseline (speedup 1.0000x reference)
"""Trainium2 Bass kernel for the binarized CNN (nn_CNN_binary_55001351193058).

Strategy (pure data-parallel over 8 NeuronCores, batch-sharded):
  - Layer 1 (real-valued conv, stride 2, k=9): dense banded matmul over the
    full input width (K=128) in float32r on the PE, producing a duplicated
    row layout that IS conv2's im2col.  x is transposed on-chip via the PE.
  - maxpool+binarize are folded into the PSUM evictions:
      pool = DVE tensor_tensor(max) over the even/odd matmul tiles,
      binarize = one DVE tensor_scalar (>= theta, -0.5) or ACT Sign(+bias).
  - Layers 2-4 + fc: exact small-integer arithmetic in bf16 matmuls
    (binarized +-1/+-0.5 activations, +-1 weights, fp32 PSUM accumulation),
    BatchNorm+Hardtanh+binarize folded into per-channel thresholds computed
    on the host in float64.  Bit-exact vs the fp32 reference except for
    conv1 accumulation-order effects.
All shapes/sharding hardcoded for B=8192, 8 cores, 1024 samples/core.
"""

import numpy as np
import ml_dtypes

import concourse.bass as bass
import concourse.mybir as mybir
import concourse.tile as tile
from concourse import bacc
from concourse.bass_utils import run_bass_kernel_spmd

F32 = mybir.dt.float32
F32R = mybir.dt.float32r
BF16 = mybir.dt.bfloat16
AF = mybir.ActivationFunctionType
ALU = mybir.AluOpType

B_TOTAL = 8192
N_CORES = 8
B_CORE = B_TOTAL // N_CORES          # 1024
NB = 64                              # samples per chunk
N_CHUNKS = B_CORE // NB              # 16
BH = NB * 6                          # 384 (b,h) columns per chunk
EPS = 1e-5

bf16 = ml_dtypes.bfloat16


# ----------------------------------------------------------------------------
# Host-side weight preparation (all in float64 where it matters)
# ----------------------------------------------------------------------------

def _sgn(w):
    return np.where(w >= 0, 1.0, -1.0)


def _threshold(g, be, m, v, bias):
    inv = g.astype(np.float64) / np.sqrt(v.astype(np.float64) + EPS)
    assert (inv > 0).all(), "BN scale must be positive for threshold folding"
    sh = be.astype(np.float64) - m.astype(np.float64) * inv
    return -bias.astype(np.float64) - sh / inv


def _check_margin(th, grid_step, name):
    # distance of each threshold from the reachable z-grid (multiples of
    # grid_step); if the reference's fp32 rounding could flip a sign the
    # margin would have to be ~1e-6 -- assert far above that.
    d = np.abs(th / grid_step - np.round(th / grid_step)) * grid_step
    if d.min() < 1e-4:
        raise AssertionError(f"threshold margin too small for {name}: {d.min()}")


def prepare_host_tensors(w1, b1, w2, b2, w3, b3, w4, b4,
                         g1, be1, m1, v1, g2, be2, m2, v2,
                         g3, be3, m3, v3, g4, be4, m4, v4, wf, bf):
    t1 = _threshold(g1, be1, m1, v1, b1)       # [32]
    t2 = _threshold(g2, be2, m2, v2, b2)       # [64]
    t3 = _threshold(g3, be3, m3, v3, b3)       # [128]
    t4 = _threshold(g4, be4, m4, v4, b4)       # [128]
    _check_margin(t2 / 2.0, 0.5, "t2")         # z2 on 0.5-grid (s1 = +-0.5)
    _check_margin(t3, 2.0, "t3")               # z3 even ints
    _check_margin(t4 / 2.0, 1.0, "t4")         # z4 ints (s3 = +-0.5)

    s1 = _sgn(w1)[:, 0, 0, :].astype(np.float32)        # [32, 9]
    s2 = _sgn(w2)[:, :, 0, :].astype(np.float32)        # [64, 32, 3]
    s3 = _sgn(w3)[:, :, 0, :].astype(np.float32)        # [128, 64, 3]
    s4 = _sgn(w4)[:, :, :, 0].astype(np.float32)        # [128, 128, 6]
    sf = _sgn(wf).astype(np.float32)                    # [10, 2048]

    # conv1 weights: 32 M-tiles (16 u-tiles x even/odd), lhsT layout [w, row]
    # row = p*32 + ci holds y1[ci, wy] with wy = 2*(2u-1+p) + half
    A1 = np.zeros((32, 128, 128), np.float32)
    for mt in range(32):
        u, half = mt // 2, mt % 2
        for p in range(4):
            w1i = 2 * u - 1 + p
            if not (0 <= w1i < 32):
                continue
            wy = 2 * w1i + half
            for k in range(9):
                wx = 2 * wy + k - 4
                if 0 <= wx < 128:
                    A1[mt, wx, p * 32:(p + 1) * 32] = s1[:, k]
    A1 = A1.reshape(32 * 128, 128)  # stacked on free dim? no: [mt*? ] see below
    # store as [128, 32*128]: partitions = w, free = (mt, row)
    A1 = A1.reshape(32, 128, 128).transpose(1, 0, 2).reshape(128, 32 * 128)

    # conv2 pair-im2col weights: lhsT [(p,ci)=128, (op,co)=128]
    W2p = np.zeros((128, 128), np.float32)
    for p in range(4):
        for op in range(2):
            k = p - op
            if 0 <= k <= 2:
                # rows p*32+ci, cols op*64+co = sigma2[co, ci, k]
                W2p[p * 32:(p + 1) * 32, op * 64:(op + 1) * 64] = s2[:, :, k].T
    # conv3 weights. Q rows: [even(w=2u): co 0..63 | odd: co 0..63]
    # tile op3=0 (w=2u):  pass a (K=128, rhs Q[:,u+1]): even->k1, odd->k2
    #                     pass b (K=64,  rhs Q[64:,u]):  odd[u-1]->k0
    # tile op3=1 (w=2u+1):pass a: even->k0, odd->k1
    #                     pass b (K=64, rhs Q[0:64,u+2]): even[u+1]->k2
    W3a = np.zeros((128, 256), np.float32)
    W3a[0:64, 0:128] = s3[:, :, 1].T
    W3a[64:128, 0:128] = s3[:, :, 2].T
    W3a[0:64, 128:256] = s3[:, :, 0].T
    W3a[64:128, 128:256] = s3[:, :, 1].T
    # K=64 passes: base partitions must match the rhs slice of Q, so store
    # the odd-pass weights in rows 64-127 and the even-pass in rows 0-63.
    W3b = np.zeros((128, 256), np.float32)
    W3b[64:128, 0:128] = s3[:, :, 0].T     # rhs Q[64:128, u]   (odd[u-1], k=0)
    W3b[0:64, 128:256] = s3[:, :, 2].T     # rhs Q[0:64, u+2]   (even[u+1], k=2)

    W4t = s4.transpose(2, 1, 0).reshape(6, 128, 128)    # [h][ci, co]
    W4t = W4t.transpose(1, 0, 2).reshape(128, 6 * 128)  # [ci, (h,co)]

    Wf = sf.reshape(10, 128, 16)                         # [j, co, w]
    Wf = Wf.transpose(1, 2, 0).reshape(128, 16 * 10)     # [co, (w,j)]
    Wf = Wf.reshape(128, 16, 10).transpose(0, 1, 2)      # keep [co][w][j]
    Wf = Wf.reshape(128, 160)

    th1 = np.tile(t1, 4).astype(np.float32).reshape(128, 1)          # rows (p,ci)
    bias2 = (-t2 / 2.0).astype(np.float32)
    bias2 = np.concatenate([bias2, bias2]).reshape(128, 1)           # (op,co)
    # th3 applies AFTER pooling (pool over op3 pairs) -> per co3 only
    th3 = t3.astype(np.float32).reshape(128, 1)
    bias4 = (-t4 / 2.0).astype(np.float32).reshape(128, 1)
    bfv = bf.astype(np.float32).reshape(10, 1)
    ident = np.eye(128, dtype=np.float32)

    return dict(
        A1=A1.astype(np.float32), A1bf=A1.astype(bf16),
        W2p=W2p.astype(bf16), W3a=W3a.astype(bf16), W3b=W3b.astype(bf16),
        W4t=W4t.astype(bf16), Wf=Wf.astype(bf16),
        th1=th1, bias2=bias2, th3=th3, bias4=bias4, bfv=bfv, ident=ident,
    )


# ----------------------------------------------------------------------------
# Bass program (identical SPMD program for each core)
# ----------------------------------------------------------------------------

def build_program():
    nc = bacc.Bacc("TRN2", target_bir_lowering=False, debug=False)

    x_d = nc.dram_tensor("x", [B_CORE, 6, 128], F32, kind="ExternalInput").ap()
    A1_d = nc.dram_tensor("A1", [128, 32 * 128], F32R, kind="ExternalInput").ap()
    A1b_d = nc.dram_tensor("A1bf", [128, 32 * 128], BF16, kind="ExternalInput").ap()
    W2_d = nc.dram_tensor("W2p", [128, 128], BF16, kind="ExternalInput").ap()
    W3a_d = nc.dram_tensor("W3a", [128, 256], BF16, kind="ExternalInput").ap()
    W3b_d = nc.dram_tensor("W3b", [128, 256], BF16, kind="ExternalInput").ap()
    W4_d = nc.dram_tensor("W4t", [128, 6 * 128], BF16, kind="ExternalInput").ap()
    Wf_d = nc.dram_tensor("Wf", [128, 160], BF16, kind="ExternalInput").ap()
    th1_d = nc.dram_tensor("th1", [128, 1], F32, kind="ExternalInput").ap()
    b2_d = nc.dram_tensor("bias2", [128, 1], F32, kind="ExternalInput").ap()
    th3_d = nc.dram_tensor("th3", [128, 1], F32, kind="ExternalInput").ap()
    b4_d = nc.dram_tensor("bias4", [128, 1], F32, kind="ExternalInput").ap()
    bf_d = nc.dram_tensor("bfv", [10, 1], F32, kind="ExternalInput").ap()
    id_d = nc.dram_tensor("ident", [128, 128], F32, kind="ExternalInput").ap()

    y_d = nc.dram_tensor("y", [B_CORE, 10], F32, kind="ExternalOutput").ap()

    with tile.TileContext(nc) as tc:
        with (
            tc.tile_pool(name="consts", bufs=1) as consts,
            tc.tile_pool(name="xin", bufs=3) as xin_pool,
            tc.tile_pool(name="xt", bufs=2) as xt_pool,
            tc.tile_pool(name="tl1", bufs=3) as tl1_pool,
            tc.tile_pool(name="s1", bufs=2) as s1_pool,
            tc.tile_pool(name="qq", bufs=2) as q_pool,
            tc.tile_pool(name="t3", bufs=3) as t3_pool,
            tc.tile_pool(name="s3", bufs=2) as s3_pool,
            tc.tile_pool(name="s4", bufs=2) as s4_pool,
            tc.tile_pool(name="oc", bufs=2) as oc_pool,
            tc.tile_pool(name="psA", bufs=3, space="PSUM") as psA_pool,   # [128,384]
            tc.tile_pool(name="psB", bufs=2, space="PSUM") as psB_pool,   # [128,512]
            tc.tile_pool(name="psC", bufs=3, space="PSUM") as psC_pool,   # [128,512]
        ):
            # --- load constants ---
            A1_s = consts.tile([128, 32 * 128], F32R)
            nc.sync.dma_start(out=A1_s, in_=A1_d)
            A1b_s = consts.tile([128, 32 * 128], BF16)
            nc.sync.dma_start(out=A1b_s, in_=A1b_d)
            W2_s = consts.tile([128, 128], BF16)
            nc.sync.dma_start(out=W2_s, in_=W2_d)
            W3a_s = consts.tile([128, 256], BF16)
            nc.sync.dma_start(out=W3a_s, in_=W3a_d)
            W3b_s = consts.tile([128, 256], BF16)
            nc.sync.dma_start(out=W3b_s, in_=W3b_d)
            W4_s = consts.tile([128, 6 * 128], BF16)
            nc.sync.dma_start(out=W4_s, in_=W4_d)
            Wf_s = consts.tile([128, 160], BF16)
            nc.sync.dma_start(out=Wf_s, in_=Wf_d)
            th1_s = consts.tile([128, 1], F32)
            nc.sync.dma_start(out=th1_s, in_=th1_d)
            b2_s = consts.tile([128, 1], F32)
            nc.sync.dma_start(out=b2_s, in_=b2_d)
            th3_s = consts.tile([128, 1], F32)
            nc.sync.dma_start(out=th3_s, in_=th3_d)
            b4_s = consts.tile([128, 1], F32)
            nc.sync.dma_start(out=b4_s, in_=b4_d)
            bf_s = consts.tile([10, 1], F32)
            nc.sync.dma_start(out=bf_s, in_=bf_d)
            id_s = consts.tile([128, 128], F32)
            nc.sync.dma_start(out=id_s, in_=id_d)

            for c in range(N_CHUNKS):
                xc = x_d[c * NB:(c + 1) * NB].rearrange("b h w -> (b h) w")

                # ---- transpose x chunk: [384 bh, 128 w] -> xT [128 w, 384] --
                ps_tr = psA_pool.tile([128, BH], F32, tag="psA")
                for t in range(3):
                    xnat = xin_pool.tile([128, 128], F32)
                    nc.sync.dma_start(out=xnat, in_=xc[128 * t:128 * (t + 1), :])
                    nc.tensor.transpose(ps_tr[:, 128 * t:128 * (t + 1)],
                                        xnat, id_s)
                xTh = xt_pool.tile([128, BH], BF16, tag="xTh")
                nc.vector.tensor_copy(xTh, ps_tr)
                xTl = xt_pool.tile([128, BH], F32R, tag="xTl")
                nc.vector.tensor_tensor(xTl, ps_tr, xTh, op=ALU.subtract)

                # ---- layer 1: 16 u-tiles x (even,odd) matmuls, pool, sign --
                s1t = s1_pool.tile([128, BH * 16], BF16)   # rows (p,ci), cols (bh,u)
                s1v = s1t.rearrange("p (bh u) -> p bh u", u=16)
                for m in range(16):
                    psa = psA_pool.tile([128, BH], F32, tag="psA")
                    psb = psA_pool.tile([128, BH], F32, tag="psA")
                    sa = slice((2 * m) * 128, (2 * m + 1) * 128)
                    sb = slice((2 * m + 1) * 128, (2 * m + 2) * 128)
                    nc.tensor.matmul(psa, A1b_s[:, sa], xTh, start=True, stop=False)
                    nc.tensor.matmul(psa, A1_s[:, sa], xTl, start=False, stop=True)
                    nc.tensor.matmul(psb, A1b_s[:, sb], xTh, start=True, stop=False)
                    nc.tensor.matmul(psb, A1_s[:, sb], xTl, start=False, stop=True)
                    sbb = tl1_pool.tile([128, BH], F32, tag="sbb")
                    nc.scalar.copy(sbb, psb)
                    tmp = tl1_pool.tile([128, BH], F32, tag="tmp")
                    nc.vector.tensor_tensor(tmp, psa, sbb, op=ALU.max)
                    nc.vector.tensor_scalar(out=s1v[:, :, m], in0=tmp,
                                            scalar1=th1_s, scalar2=0.5,
                                            op0=ALU.is_ge, op1=ALU.subtract)
                # zero the pad slots: (p=0, u=0) -> w=-1 ; (p=3, u=15) -> w=32
                nc.gpsimd.memset(s1v[0:32, :, 0], 0.0)
                nc.gpsimd.memset(s1v[96:128, :, 15], 0.0)

                # ---- layer 2: one K=128 matmul per 512-col sub-chunk -------
                qt = q_pool.tile([128, BH * 18], BF16)     # cols (bh, u') u'=u+1
                qv = qt.rearrange("p (bh u) -> p bh u", u=18)
                nc.gpsimd.memset(qv[:, :, 0], 0.0)
                nc.gpsimd.memset(qv[:, :, 17], 0.0)
                for s in range(12):
                    ps2 = psB_pool.tile([128, 512], F32, tag="psB")
                    nc.tensor.matmul(ps2, W2_s,
                                     s1t[:, 512 * s:512 * (s + 1)],
                                     start=True, stop=True)
                    nc.scalar.activation(
                        qv[:, 32 * s:32 * (s + 1), 1:17],
                        ps2.rearrange("p (a b) -> p a b", b=16),
                        AF.Sign, bias=b2_s)

                # ---- layer 3: 2 M-tiles x (K=128 + K=64) per sub-chunk -----
                s3t = s3_pool.tile([128, BH * 16], BF16)   # [co3, (bh,u)]
                for s in range(12):
                    p3a = psC_pool.tile([128, 512], F32, tag="psC")
                    p3b = psC_pool.tile([128, 512], F32, tag="psC")
                    q_mid = qv[:, 32 * s:32 * (s + 1), 1:17]
                    nc.tensor.matmul(p3a, W3a_s[:, 0:128], q_mid, start=True,
                                     stop=False)
                    nc.tensor.matmul(p3a, W3b_s[64:128, 0:128],
                                     qv[64:128, 32 * s:32 * (s + 1), 0:16],
                                     start=False, stop=True)
                    nc.tensor.matmul(p3b, W3a_s[:, 128:256], q_mid, start=True,
                                     stop=False)
                    nc.tensor.matmul(p3b, W3b_s[0:64, 128:256],
                                     qv[0:64, 32 * s:32 * (s + 1), 2:18],
                                     start=False, stop=True)
                    sb3b = t3_pool.tile([128, 512], BF16, tag="sb3b")
                    nc.scalar.copy(sb3b, p3b)
                    tmp3 = t3_pool.tile([128, 512], BF16, tag="tmp3")
                    nc.vector.tensor_tensor(tmp3, p3a, sb3b, op=ALU.max)
                    nc.vector.tensor_scalar(out=s3t[:, 512 * s:512 * (s + 1)],
                                            in0=tmp3, scalar1=th3_s, scalar2=0.5,
                                            op0=ALU.is_ge, op1=ALU.subtract)

                # ---- layer 4: contract (ci, h); 2 N-halves of 512 ----------
                s4t = s4_pool.tile([128, NB * 16], BF16)   # [co4, (b,w)]
                s3v = s3t.rearrange("p (b h u) -> p b h u", h=6, u=16)
                for half in range(2):
                    bsl = slice(32 * half, 32 * (half + 1))
                    ps4 = psB_pool.tile([128, 512], F32, tag="psB")
                    for h in range(6):
                        nc.tensor.matmul(
                            ps4, W4_s[:, 128 * h:128 * (h + 1)],
                            s3v[:, bsl, h, :],
                            start=(h == 0), stop=(h == 5))
                    nc.scalar.activation(s4t[:, 512 * half:512 * (half + 1)],
                                         ps4, AF.Sign, bias=b4_s)

                # ---- fc ----------------------------------------------------
                s4v = s4t.rearrange("p (b w) -> p b w", w=16)
                psf = psA_pool.tile([10, 64], F32, tag="psA")
                for w in range(16):
                    nc.tensor.matmul(psf, Wf_s[:, 10 * w:10 * (w + 1)],
                                     s4v[:, :, w:w + 1], start=(w == 0), stop=(w == 15))
                outc = oc_pool.tile([10, NB], F32)
                nc.vector.tensor_scalar_add(outc, psf, bf_s)
                nc.sync.dma_start(
                    out=y_d[c * NB:(c + 1) * NB, :].rearrange("b j -> j b"),
                    in_=outc)

    nc.compile()
    return nc


_PROGRAM = None


def _get_program():
    global _PROGRAM
    if _PROGRAM is None:
        _PROGRAM = build_program()
    return _PROGRAM


def run(trace=False, **inputs):
    inputs = {k: np.asarray(v) for k, v in inputs.items()}
    consts = prepare_host_tensors(
        **{k: inputs[k] for k in
           ("w1", "b1", "w2", "b2", "w3", "b3", "w4", "b4",
            "g1", "be1", "m1", "v1", "g2", "be2", "m2", "v2",
            "g3", "be3", "m3", "v3", "g4", "be4", "m4", "v4", "wf", "bf")})
    x = inputs["x"].astype(np.float32)           # [8192, 1, 6, 128]
    nc = _get_program()
    in_maps = []
    for k in range(N_CORES):
        m = {"x": np.ascontiguousarray(x[k * B_CORE:(k + 1) * B_CORE, 0])}
        m.update(consts)
        in_maps.append(m)
    res = run_bass_kernel_spmd(nc, in_maps, list(range(N_CORES)), trace=trace)
    y = np.concatenate([r["y"] for r in res.results], axis=0)
    return y.astype(np.float32), res


def kernel(**inputs):
    y, _ = run(trace=False, **inputs)
    return y



# revision 11
# speedup vs baseline: 1.4545x; 1.4545x over previous
"""Trainium2 Bass kernel for the binarized CNN (nn_CNN_binary_55001351193058).

v2 — restructured from the baseline for engine balance and PE density:
  - Host pre-transposes x and performs the hi/lo bf16 split (xh = bf16(xT),
    xl = bf16(xT - xh)); removes on-chip PE transposes and the f32r pass.
    Both L1 passes share one bf16 A1 (one LDWEIGHTS per M-tile).
  - Activations use a {0,1} convention for s1/s3 (binarize = is_ge) so the
    pool+binarize eviction is two fused ops: tensor_scalar on one engine and
    scalar_tensor_tensor (is_ge then max) on another, spread across DVE and
    GPSIMD.  Out-of-range conv2 taps are handled by 0.5-valued pad cells
    (sigma*(2*0.5-1) = 0), memset once, never rewritten -> no per-chunk
    memsets and no edge-specific biases.  q/s4 use +-1 via ACT Sign.
  - L3 uses fp8e4 DoubleRow matmuls (2 k-tiles per pass, exact for +-1/0
    values): one MM per output parity instead of two.  L4 and the fc layer
    are DoubleRow over h-pairs / w-pairs.
  - Software-pipelined emission: chunk c's L1 matmuls are emitted before
    chunk c-1's L2/L3/L4/fc so the PE never waits on an eviction of the
    chunk it just produced.  PSUM pools are shared across phases (8 banks).
All shapes hardcoded for B=8192, 8 cores, 1024 samples/core.
"""

import numpy as np
import ml_dtypes

import concourse.bass as bass
import concourse.mybir as mybir
import concourse.tile as tile
from concourse import bacc
from concourse.ap import AP
from concourse.bass_utils import run_bass_kernel_spmd

F32 = mybir.dt.float32
BF16 = mybir.dt.bfloat16
F16 = mybir.dt.float16
FP8 = mybir.dt.float8e4
AF = mybir.ActivationFunctionType
ALU = mybir.AluOpType
DR = mybir.MatmulPerfMode.DoubleRow

B_TOTAL = 8192
N_CORES = 8
B_CORE = B_TOTAL // N_CORES          # 1024
NB = 64                              # samples per chunk
BH = NB * 6                          # 384 (b,h) columns per chunk
NBLK = 16                            # s-blocks per chunk (24 bh x 16 u each)
BHB = BH // NBLK                     # 24 bh per block (4 samples)
EPS = 1e-5

bf16 = ml_dtypes.bfloat16
f8 = ml_dtypes.float8_e4m3


# ----------------------------------------------------------------------------
# Host-side weight preparation (float64 where it matters)
# ----------------------------------------------------------------------------

def _sgn(w):
    return np.where(w >= 0, 1.0, -1.0)


def _threshold(g, be, m, v, bias):
    inv = g.astype(np.float64) / np.sqrt(v.astype(np.float64) + EPS)
    assert (inv > 0).all(), "BN scale must be positive for threshold folding"
    sh = be.astype(np.float64) - m.astype(np.float64) * inv
    return -bias.astype(np.float64) - sh / inv


def _check_margin(x, grid_step, name):
    d = np.abs(x / grid_step - np.round(x / grid_step)) * grid_step
    if d.min() < 1e-4:
        raise AssertionError(f"threshold margin too small for {name}: {d.min()}")


def prepare_host_tensors(w1, b1, w2, b2, w3, b3, w4, b4,
                         g1, be1, m1, v1, g2, be2, m2, v2,
                         g3, be3, m3, v3, g4, be4, m4, v4, wf, bf):
    t1 = _threshold(g1, be1, m1, v1, b1)       # [32]
    t2 = _threshold(g2, be2, m2, v2, b2)       # [64]
    t3 = _threshold(g3, be3, m3, v3, b3)       # [128]
    t4 = _threshold(g4, be4, m4, v4, b4)       # [128]

    s1 = _sgn(w1)[:, 0, 0, :].astype(np.float64)        # [32, 9]
    s2 = _sgn(w2)[:, :, 0, :].astype(np.float64)        # [64, 32, 3]
    s3 = _sgn(w3)[:, :, 0, :].astype(np.float64)        # [128, 64, 3]
    s4 = _sgn(w4)[:, :, :, 0].astype(np.float64)        # [128, 128, 6]
    sf = _sgn(wf).astype(np.float64)                    # [10, 2048]

    # conv1 banded weights: 32 M-tiles (16 u-tiles x even/odd); lhsT layout
    # [wx, (mt, row)] with row = p*32 + ci holding y1[ci, wy],
    # wy = 2*(2u-1+p) + half.  Stride-2 conv, 9 taps, pad 4.
    A1 = np.zeros((32, 128, 128), np.float64)
    for mt in range(32):
        u, half = mt // 2, mt % 2
        for p in range(4):
            w1i = 2 * u - 1 + p
            if not (0 <= w1i < 32):
                continue
            wy = 2 * w1i + half
            for k in range(9):
                wx = 2 * wy + k - 4
                if 0 <= wx < 128:
                    A1[mt, wx, p * 32:(p + 1) * 32] = s1[:, k]
    A1 = A1.transpose(1, 0, 2).reshape(128, 32 * 128)

    th1 = np.tile(t1, 4).astype(np.float32).reshape(128, 1)   # rows (p,ci)

    # conv2 pair-im2col weights: lhsT [(p,ci)=128, (op,co)=128]
    W2p = np.zeros((128, 128), np.float64)
    for p in range(4):
        for op in range(2):
            k = p - op
            if 0 <= k <= 2:
                W2p[p * 32:(p + 1) * 32, op * 64:(op + 1) * 64] = s2[:, :, k].T
    # {0,1} inputs with 0.5-valued pads: z2_ref = 2*psum - S2c
    S2c = W2p.sum(axis=0)                                  # [128] per (op,co)
    t2col = np.concatenate([t2, t2])                       # [(op,co)]
    b2p = (-(t2col + S2c) / 2.0)
    _check_margin(-b2p, 0.5, "b2p")                        # psum on 0.5-grid
    b2p = b2p.astype(np.float32).reshape(128, 1)

    # conv3 weights, DoubleRow pairs (j = u'-offset relative to each MM base)
    W3a = np.zeros((128, 256), np.float64)
    W3a[0:64, 0:128] = s3[:, :, 1].T
    W3a[64:128, 0:128] = s3[:, :, 2].T
    W3a[0:64, 128:256] = s3[:, :, 0].T
    W3a[64:128, 128:256] = s3[:, :, 1].T
    W3b = np.zeros((128, 256), np.float64)
    W3b[64:128, 0:128] = s3[:, :, 0].T     # odd rows, k=0 (u'-offset 0 for p3a)
    W3b[0:64, 128:256] = s3[:, :, 2].T     # even rows, k=2 (u'-offset 2 for p3b)
    # p3a (even outputs w3=2u):  j=0 -> u'=u   (W3b cols 0:128)
    #                            j=1 -> u'=u+1 (W3a cols 0:128)
    # p3b (odd outputs w3=2u+1): j=0 -> u'=u+1 (W3a cols 128:256)
    #                            j=1 -> u'=u+2 (W3b cols 128:256)
    W3Adr = np.concatenate([W3b[:, 0:128], W3a[:, 0:128]], axis=1)
    W3Bdr = np.concatenate([W3a[:, 128:256], W3b[:, 128:256]], axis=1)
    th3 = t3.astype(np.float32).reshape(128, 1)
    _check_margin(t3, 1.0, "t3")                           # psum3 integer grid

    # conv4 (6,1) DoubleRow over h-pairs: per a: [ci, j, co4]
    W4dr = np.zeros((128, 3 * 256), np.float64)
    for a in range(3):
        for j in range(2):
            W4dr[:, a * 256 + j * 128:(a * 256 + (j + 1) * 128)] = \
                s4[:, :, 2 * a + j].T
    # {0,1} s3: z4_ref = 2*psum - S4c
    S4c = s4.sum(axis=(1, 2))                              # [128]
    b4p = (-(t4 + S4c) / 2.0)
    _check_margin(-b4p, 1.0, "b4p")                        # psum4 integer grid
    b4p = b4p.astype(np.float32).reshape(128, 1)

    # fc DoubleRow over w-pairs: per a: [co4, j, jo]
    sfr = sf.reshape(10, 128, 16)
    Wfdr = np.zeros((128, 8 * 32), np.float64)
    for a in range(8):
        for j in range(2):
            Wfdr[:, a * 32 + j * 16:a * 32 + j * 16 + 10] = \
                sfr[:, :, 2 * a + j].T
    bfv = bf.astype(np.float32).reshape(10, 1)

    return dict(
        A1=A1.astype(np.float16), th1=th1, th1n=-th1, th3n=-t3.astype(np.float32).reshape(128, 1),
        W2p=W2p.astype(bf16), b2p=b2p,
        W3Adr=W3Adr.astype(f8), W3Bdr=W3Bdr.astype(f8), th3=th3,
        W4dr=W4dr.astype(f8), b4p=b4p,
        Wfdr=Wfdr.astype(f8), bfv=bfv,
    )


def prepare_x(x_core):
    """x_core [B_CORE, 6, 128] fp32 -> (xh, xl) [128, B_CORE*6] bf16."""
    xT = np.ascontiguousarray(x_core.reshape(-1, 128).T.astype(np.float32))
    xh = xT.astype(np.float16)
    xl = (xT - xh.astype(np.float32)).astype(np.float16)
    return xh, xl


# ----------------------------------------------------------------------------
# Bass program (identical SPMD program per core)
# ----------------------------------------------------------------------------

def _ap(base, off, dims):
    """Raw strided AP view into a tile, dims = [(stride, n), ...] after the
    partition dim (which is copied from base)."""
    p = list(base.ap)[0]
    return AP(base.tensor, base.offset + off, [list(p)] + [list(d) for d in dims])


def build_program(n_chunks=B_CORE // NB):
    nc = bacc.Bacc("TRN2", target_bir_lowering=False, debug=False)
    bcore = n_chunks * NB

    xh_d = nc.dram_tensor("xh", [128, bcore * 6], F16, kind="ExternalInput").ap()
    xl_d = nc.dram_tensor("xl", [128, bcore * 6], F16, kind="ExternalInput").ap()
    A1_d = nc.dram_tensor("A1", [128, 32 * 128], F16, kind="ExternalInput").ap()
    th1_d = nc.dram_tensor("th1", [128, 1], F32, kind="ExternalInput").ap()
    th1n_d = nc.dram_tensor("th1n", [128, 1], F32, kind="ExternalInput").ap()
    th3n_d = nc.dram_tensor("th3n", [128, 1], F32, kind="ExternalInput").ap()
    W2_d = nc.dram_tensor("W2p", [128, 128], BF16, kind="ExternalInput").ap()
    b2_d = nc.dram_tensor("b2p", [128, 1], F32, kind="ExternalInput").ap()
    W3A_d = nc.dram_tensor("W3Adr", [128, 256], FP8, kind="ExternalInput").ap()
    W3B_d = nc.dram_tensor("W3Bdr", [128, 256], FP8, kind="ExternalInput").ap()
    th3_d = nc.dram_tensor("th3", [128, 1], F32, kind="ExternalInput").ap()
    W4_d = nc.dram_tensor("W4dr", [128, 3 * 256], FP8, kind="ExternalInput").ap()
    b4_d = nc.dram_tensor("b4p", [128, 1], F32, kind="ExternalInput").ap()
    Wf_d = nc.dram_tensor("Wfdr", [128, 8 * 32], FP8, kind="ExternalInput").ap()
    bf_d = nc.dram_tensor("bfv", [10, 1], F32, kind="ExternalInput").ap()

    y_d = nc.dram_tensor("y", [bcore, 10], F32, kind="ExternalOutput").ap()
    import os
    dbg = os.environ.get("KDEBUG") == "1"
    if dbg:
        dbg_s1 = nc.dram_tensor("dbg_s1", [128, BH * 16], F32, kind="ExternalOutput").ap()
        dbg_q = nc.dram_tensor("dbg_q", [128, BH * 18], F32, kind="ExternalOutput").ap()
        dbg_s3 = nc.dram_tensor("dbg_s3", [128, NB * 96], F32, kind="ExternalOutput").ap()
        dbg_s4 = nc.dram_tensor("dbg_s4", [128, 2 * NB * 16], F32, kind="ExternalOutput").ap()

    with tile.TileContext(nc) as tc:
        with (
            tc.tile_pool(name="consts", bufs=1) as consts,
            tc.tile_pool(name="xin", bufs=2) as xin_pool,
            tc.tile_pool(name="bt", bufs=4) as bt_pool,       # B' binarize tmps
            tc.tile_pool(name="oc", bufs=2) as oc_pool,
            tc.tile_pool(name="pA", bufs=5, space="PSUM") as pA,   # L1+L3 384
            tc.tile_pool(name="pB", bufs=2, space="PSUM") as pB,   # L2+L4 512
            tc.tile_pool(name="pF", bufs=1, space="PSUM") as pF,   # fc
        ):
            # --- constants ---
            A1_s = consts.tile([128, 32 * 128], F16)
            nc.sync.dma_start(out=A1_s, in_=A1_d)
            th1_s = consts.tile([128, 1], F32)
            nc.sync.dma_start(out=th1_s, in_=th1_d)
            th1n_s = consts.tile([128, 1], F32)
            nc.sync.dma_start(out=th1n_s, in_=th1n_d)
            th3n_s = consts.tile([128, 1], F32)
            nc.sync.dma_start(out=th3n_s, in_=th3n_d)
            W2_s = consts.tile([128, 128], BF16)
            nc.sync.dma_start(out=W2_s, in_=W2_d)
            b2_s = consts.tile([128, 1], F32)
            nc.sync.dma_start(out=b2_s, in_=b2_d)
            W3A_s = consts.tile([128, 256], FP8)
            nc.sync.dma_start(out=W3A_s, in_=W3A_d)
            W3B_s = consts.tile([128, 256], FP8)
            nc.sync.dma_start(out=W3B_s, in_=W3B_d)
            th3_s = consts.tile([128, 1], F32)
            nc.sync.dma_start(out=th3_s, in_=th3_d)
            W4_s = consts.tile([128, 3 * 256], FP8)
            nc.sync.dma_start(out=W4_s, in_=W4_d)
            b4_s = consts.tile([128, 1], F32)
            nc.sync.dma_start(out=b4_s, in_=b4_d)
            Wf_s = consts.tile([128, 8 * 32], FP8)
            nc.sync.dma_start(out=Wf_s, in_=Wf_d)
            bf_s = consts.tile([10, 1], F32)
            nc.sync.dma_start(out=bf_s, in_=bf_d)

            W3A_v = W3A_s.rearrange("p (j m) -> p j m", j=2)
            W3B_v = W3B_s.rearrange("p (j m) -> p j m", j=2)
            W4_v = W4_s.rearrange("p (a j m) -> p a j m", a=3, j=2)
            Wf_v = Wf_s.rearrange("p (a j m) -> p a j m", a=8, j=2, m=16)

            # --- persistent double-buffered activations ---
            s1t = [consts.tile([128, BH * 16], BF16, name=f"s1t{i}") for i in range(2)]
            qt = [consts.tile([128, BH * 18], FP8, name=f"qt{i}") for i in range(2)]
            s3t = [consts.tile([128, NB * 96], FP8, name=f"s3t{i}") for i in range(2)]  # (b,u,h)
            s4t = [consts.tile([128, 2 * NB * 16], FP8, name=f"s4t{i}") for i in range(2)]

            # one-time pads: s1 pad cells = 0.5 (so 2*s-1 = 0), q pads = 0
            for t in s1t:
                v = t.rearrange("p (bh u) -> p bh u", u=16)
                nc.gpsimd.memset(v[0:32, :, 0], 0.5)
                nc.gpsimd.memset(v[96:128, :, 15], 0.5)
            for t in qt:
                nc.gpsimd.memset(t[:, 0:BH], 0.0)
                nc.gpsimd.memset(t[:, 17 * BH:18 * BH], 0.0)
            if dbg:
                for t in s4t:
                    nc.gpsimd.memset(t, 0.0)

            eng = [nc.vector, nc.gpsimd]

            for it in range(n_chunks + 1):
                # ================= L1 for chunk c = it =================
                if it < n_chunks:
                    c = it
                    xh_s = xin_pool.tile([128, BH], F16, tag="xh")
                    nc.sync.dma_start(out=xh_s, in_=xh_d[:, c * BH:(c + 1) * BH])
                    xl_s = xin_pool.tile([128, BH], F16, tag="xl")
                    nc.sync.dma_start(out=xl_s, in_=xl_d[:, c * BH:(c + 1) * BH])
                    s1v = s1t[c % 2].rearrange("p (bh u) -> p bh u", u=16)
                    for m in range(16):
                        psa = pA.tile([128, BH], F32, tag="pp")
                        psb = pA.tile([128, BH], F32, tag="pp")
                        sa = slice((2 * m) * 128, (2 * m + 1) * 128)
                        sb = slice((2 * m + 1) * 128, (2 * m + 2) * 128)
                        nc.tensor.matmul(psa, A1_s[:, sa], xh_s, start=True, stop=False)
                        nc.tensor.matmul(psa, A1_s[:, sa], xl_s, start=False, stop=True)
                        nc.tensor.matmul(psb, A1_s[:, sb], xh_s, start=True, stop=False)
                        nc.tensor.matmul(psb, A1_s[:, sb], xl_s, start=False, stop=True)
                        b1p = bt_pool.tile([128, BH], BF16, tag="b1")
                        if m % 8 != 7:
                            # ACT: +-1; max({0,1}, +-1) still yields {0,1}
                            nc.scalar.activation(b1p, psb, AF.Sign, bias=th1n_s)
                        else:
                            nc.vector.tensor_scalar(out=b1p, in0=psb, scalar1=th1_s,
                                                    scalar2=None, op0=ALU.is_ge)
                        if m == 0:
                            slices = [slice(32, 64), slice(64, 128)]
                        elif m == 15:
                            slices = [slice(0, 96)]
                        else:
                            slices = [slice(0, 128)]
                        for rs in slices:
                            nc.vector.scalar_tensor_tensor(
                                out=s1v[rs, :, m], in0=psa[rs], scalar=th1_s[rs],
                                in1=b1p[rs], op0=ALU.is_ge, op1=ALU.max)

                # ============ L2/L3/L4/fc for chunk c = it-1 ============
                if it >= 1:
                    c = it - 1
                    s1c = s1t[c % 2]
                    qb = qt[c % 2]
                    s3c = s3t[c % 2]
                    # ---- L2: 12 blocks of (32 bh x 16 u) ----
                    for s in range(12):
                        ps2 = pB.tile([128, 512], F32, tag="pb")
                        nc.tensor.matmul(ps2, W2_s,
                                         s1c[:, 512 * s:512 * (s + 1)],
                                         start=True, stop=True)
                        nc.scalar.activation(
                            _ap(qb, BH + 32 * s, [(1, 32), (BH, 16)]),
                            ps2.rearrange("p (a b) -> p a b", b=16),
                            AF.Sign, bias=b2_s)
                    # ---- L3: DoubleRow, 12 raster blocks of 512 per parity ----
                    for k in range(12):
                        p3a = pA.tile([128, 512], F32, tag="pp")
                        p3b = pA.tile([128, 512], F32, tag="pp")
                        rha = _ap(qb, 512 * k, [(BH, 2), (1, 512)])
                        rhb = _ap(qb, 512 * k + BH, [(BH, 2), (1, 512)])
                        nc.tensor.matmul(p3a, W3A_v, rha, start=True, stop=True,
                                         perf_mode=DR)
                        nc.tensor.matmul(p3b, W3B_v, rhb, start=True, stop=True,
                                         perf_mode=DR)
                        b3p = bt_pool.tile([128, 512], BF16, tag="b3")
                        if k % 6 != 5:
                            nc.scalar.activation(b3p, p3b, AF.Sign, bias=th3n_s)
                        else:
                            nc.vector.tensor_scalar(out=b3p, in0=p3b, scalar1=th3_s,
                                                    scalar2=None, op0=ALU.is_ge)
                        nc.vector.scalar_tensor_tensor(
                            out=s3c[:, 512 * k:512 * (k + 1)], in0=p3a,
                            scalar=th3_s, in1=b3p, op0=ALU.is_ge, op1=ALU.max)
                    # ---- L4: DoubleRow over h-pairs, 2 b-halves ----
                    ps4 = [pB.tile([128, 512], F32, tag="pb", name=f"ps4_{half}") for half in range(2)]
                    for a in range(3):
                        for half in range(2):
                            rh4 = _ap(s3c, half * 3072 + 2 * a,
                                      [(1, 2), (6, 512)])
                            nc.tensor.matmul(ps4[half], W4_v[:, a], rh4,
                                             start=(a == 0), stop=(a == 2),
                                             perf_mode=DR)
                    s4g = s4t[(c // 2) % 2]
                    cc = c % 2
                    for half in range(2):
                        nc.scalar.activation(
                            s4g[:, cc * 1024 + half * 512:cc * 1024 + (half + 1) * 512],
                            ps4[half], AF.Sign, bias=b4_s)
                    # ---- fc: DoubleRow over w-pairs, once per 2 chunks ----
                    if c % 2 == 1 or c == n_chunks - 1:
                        ncc = (c % 2) + 1            # chunks in this group
                        nn = ncc * NB
                        psf = pF.tile([16, 2 * NB], F32, tag="pf")
                        for cc2 in range(ncc):
                            for a in range(8):
                                rhf = _ap(s4g, cc2 * 1024 + 128 * a,
                                          [(64, 2), (1, 64)])
                                nc.tensor.matmul(psf[:, cc2 * 64:(cc2 + 1) * 64],
                                                 Wf_v[:, a], rhf,
                                                 start=(a == 0), stop=(a == 7),
                                                 perf_mode=DR, skip_group_check=True)
                        outc = oc_pool.tile([10, 2 * NB], F32)
                        nc.vector.tensor_scalar_add(outc[:, 0:nn], psf[0:10, 0:nn], bf_s)
                        g0 = (c // 2) * 2 * NB
                        nc.sync.dma_start(
                            out=y_d[g0:g0 + nn, :].rearrange("b j -> j b"),
                            in_=outc[:, 0:nn])

            if dbg:
                d1 = consts.tile([128, BH * 16], F32)
                nc.vector.tensor_copy(d1, s1t[0])
                nc.sync.dma_start(out=dbg_s1, in_=d1)
                d2 = consts.tile([128, BH * 18], F32)
                nc.vector.tensor_copy(d2, qt[0])
                nc.sync.dma_start(out=dbg_q, in_=d2)
                d3 = consts.tile([128, NB * 96], F32)
                nc.vector.tensor_copy(d3, s3t[0])
                nc.sync.dma_start(out=dbg_s3, in_=d3)
                d4 = consts.tile([128, 2 * NB * 16], F32)
                nc.vector.tensor_copy(d4, s4t[0])
                nc.sync.dma_start(out=dbg_s4, in_=d4)

    nc.compile()
    return nc


_PROGRAM = None


def _get_program():
    global _PROGRAM
    if _PROGRAM is None:
        _PROGRAM = build_program()
    return _PROGRAM


def run(trace=False, **inputs):
    inputs = {k: np.asarray(v) for k, v in inputs.items()}
    consts = prepare_host_tensors(
        **{k: inputs[k] for k in
           ("w1", "b1", "w2", "b2", "w3", "b3", "w4", "b4",
            "g1", "be1", "m1", "v1", "g2", "be2", "m2", "v2",
            "g3", "be3", "m3", "v3", "g4", "be4", "m4", "v4", "wf", "bf")})
    x = inputs["x"].astype(np.float32)           # [8192, 1, 6, 128]
    nc = _get_program()
    in_maps = []
    for k in range(N_CORES):
        xh, xl = prepare_x(x[k * B_CORE:(k + 1) * B_CORE, 0])
        m = {"xh": xh, "xl": xl}
        m.update(consts)
        in_maps.append(m)
    res = run_bass_kernel_spmd(nc, in_maps, list(range(N_CORES)), trace=trace)
    y = np.concatenate([r["y"] for r in res.results], axis=0)
    return y.astype(np.float32), res


def kernel(**inputs):
    y, _ = run(trace=False, **inputs)
    return y


# revision 12
# speedup vs baseline: 2.8161x; 1.9361x over previous
"""Trainium2 Bass kernel for the binarized CNN (nn_CNN_binary_55001351193058).

v2 — restructured from the baseline for engine balance and PE density:
  - Host pre-transposes x and performs the hi/lo bf16 split (xh = bf16(xT),
    xl = bf16(xT - xh)); removes on-chip PE transposes and the f32r pass.
    Both L1 passes share one bf16 A1 (one LDWEIGHTS per M-tile).
  - Activations use a {0,1} convention for s1/s3 (binarize = is_ge) so the
    pool+binarize eviction is two fused ops: tensor_scalar on one engine and
    scalar_tensor_tensor (is_ge then max) on another, spread across DVE and
    GPSIMD.  Out-of-range conv2 taps are handled by 0.5-valued pad cells
    (sigma*(2*0.5-1) = 0), memset once, never rewritten -> no per-chunk
    memsets and no edge-specific biases.  q/s4 use +-1 via ACT Sign.
  - L3 uses fp8e4 DoubleRow matmuls (2 k-tiles per pass, exact for +-1/0
    values): one MM per output parity instead of two.  L4 and the fc layer
    are DoubleRow over h-pairs / w-pairs.
  - Software-pipelined emission: chunk c's L1 matmuls are emitted before
    chunk c-1's L2/L3/L4/fc so the PE never waits on an eviction of the
    chunk it just produced.  PSUM pools are shared across phases (8 banks).
All shapes hardcoded for B=8192, 8 cores, 1024 samples/core.
"""

import numpy as np
import ml_dtypes

import concourse.bass as bass
import concourse.mybir as mybir
import concourse.tile as tile
from concourse import bacc
from concourse.ap import AP
from concourse.bass_utils import run_bass_kernel_spmd

F32 = mybir.dt.float32
BF16 = mybir.dt.bfloat16
F16 = mybir.dt.float16
FP8 = mybir.dt.float8e4
AF = mybir.ActivationFunctionType
ALU = mybir.AluOpType
DR = mybir.MatmulPerfMode.DoubleRow

B_TOTAL = 8192
N_CORES = 8
B_CORE = B_TOTAL // N_CORES          # 1024
NB = 64                              # samples per chunk
BH = NB * 6                          # 384 (b,h) columns per chunk
NBLK = 16                            # s-blocks per chunk (24 bh x 16 u each)
BHB = BH // NBLK                     # 24 bh per block (4 samples)
EPS = 1e-5

bf16 = ml_dtypes.bfloat16
f8 = ml_dtypes.float8_e4m3


# ----------------------------------------------------------------------------
# Host-side weight preparation (float64 where it matters)
# ----------------------------------------------------------------------------

def _sgn(w):
    return np.where(w >= 0, 1.0, -1.0)


def _threshold(g, be, m, v, bias):
    inv = g.astype(np.float64) / np.sqrt(v.astype(np.float64) + EPS)
    assert (inv > 0).all(), "BN scale must be positive for threshold folding"
    sh = be.astype(np.float64) - m.astype(np.float64) * inv
    return -bias.astype(np.float64) - sh / inv


def _check_margin(x, grid_step, name):
    d = np.abs(x / grid_step - np.round(x / grid_step)) * grid_step
    if d.min() < 1e-4:
        raise AssertionError(f"threshold margin too small for {name}: {d.min()}")


def prepare_host_tensors(w1, b1, w2, b2, w3, b3, w4, b4,
                         g1, be1, m1, v1, g2, be2, m2, v2,
                         g3, be3, m3, v3, g4, be4, m4, v4, wf, bf):
    t1 = _threshold(g1, be1, m1, v1, b1)       # [32]
    t2 = _threshold(g2, be2, m2, v2, b2)       # [64]
    t3 = _threshold(g3, be3, m3, v3, b3)       # [128]
    t4 = _threshold(g4, be4, m4, v4, b4)       # [128]

    s1 = _sgn(w1)[:, 0, 0, :].astype(np.float64)        # [32, 9]
    s2 = _sgn(w2)[:, :, 0, :].astype(np.float64)        # [64, 32, 3]
    s3 = _sgn(w3)[:, :, 0, :].astype(np.float64)        # [128, 64, 3]
    s4 = _sgn(w4)[:, :, :, 0].astype(np.float64)        # [128, 128, 6]
    sf = _sgn(wf).astype(np.float64)                    # [10, 2048]

    # conv1 banded weights: 32 M-tiles (16 u-tiles x even/odd); lhsT layout
    # [wx, (mt, row)] with row = p*32 + ci holding y1[ci, wy],
    # wy = 2*(2u-1+p) + half.  Stride-2 conv, 9 taps, pad 4.
    A1 = np.zeros((32, 128, 128), np.float64)
    for mt in range(32):
        u, half = mt // 2, mt % 2
        for p in range(4):
            w1i = 2 * u - 1 + p
            if not (0 <= w1i < 32):
                continue
            wy = 2 * w1i + half
            for k in range(9):
                wx = 2 * wy + k - 4
                if 0 <= wx < 128:
                    A1[mt, wx, p * 32:(p + 1) * 32] = s1[:, k]
    A1 = A1.transpose(1, 0, 2).reshape(128, 32 * 128)

    th1 = np.tile(t1, 4).astype(np.float32).reshape(128, 1)   # rows (p,ci)

    # conv2 pair-im2col weights: lhsT [(p,ci)=128, (op,co)=128]
    W2p = np.zeros((128, 128), np.float64)
    for p in range(4):
        for op in range(2):
            k = p - op
            if 0 <= k <= 2:
                W2p[p * 32:(p + 1) * 32, op * 64:(op + 1) * 64] = s2[:, :, k].T
    # {0,1} inputs with 0.5-valued pads: z2_ref = 2*psum - S2c
    S2c = W2p.sum(axis=0)                                  # [128] per (op,co)
    t2col = np.concatenate([t2, t2])                       # [(op,co)]
    b2p = (-(t2col + S2c) / 2.0)
    _check_margin(-b2p, 0.5, "b2p")                        # psum on 0.5-grid
    b2p = b2p.astype(np.float32).reshape(128, 1)

    # conv3 weights, DoubleRow pairs (j = u'-offset relative to each MM base)
    W3a = np.zeros((128, 256), np.float64)
    W3a[0:64, 0:128] = s3[:, :, 1].T
    W3a[64:128, 0:128] = s3[:, :, 2].T
    W3a[0:64, 128:256] = s3[:, :, 0].T
    W3a[64:128, 128:256] = s3[:, :, 1].T
    W3b = np.zeros((128, 256), np.float64)
    W3b[64:128, 0:128] = s3[:, :, 0].T     # odd rows, k=0 (u'-offset 0 for p3a)
    W3b[0:64, 128:256] = s3[:, :, 2].T     # even rows, k=2 (u'-offset 2 for p3b)
    # p3a (even outputs w3=2u):  j=0 -> u'=u   (W3b cols 0:128)
    #                            j=1 -> u'=u+1 (W3a cols 0:128)
    # p3b (odd outputs w3=2u+1): j=0 -> u'=u+1 (W3a cols 128:256)
    #                            j=1 -> u'=u+2 (W3b cols 128:256)
    W3Adr = np.concatenate([W3b[:, 0:128], W3a[:, 0:128]], axis=1)
    W3Bdr = np.concatenate([W3a[:, 128:256], W3b[:, 128:256]], axis=1)
    th3 = t3.astype(np.float32).reshape(128, 1)
    _check_margin(t3, 1.0, "t3")                           # psum3 integer grid

    # conv4 (6,1) DoubleRow over h-pairs: per a: [ci, j, co4]
    W4dr = np.zeros((128, 3 * 256), np.float64)
    for a in range(3):
        for j in range(2):
            W4dr[:, a * 256 + j * 128:(a * 256 + (j + 1) * 128)] = \
                s4[:, :, 2 * a + j].T
    # {0,1} s3: z4_ref = 2*psum - S4c
    S4c = s4.sum(axis=(1, 2))                              # [128]
    b4p = (-(t4 + S4c) / 2.0)
    _check_margin(-b4p, 1.0, "b4p")                        # psum4 integer grid
    b4p = b4p.astype(np.float32).reshape(128, 1)

    # fc DoubleRow over w-pairs: per a: [co4, j, jo]
    sfr = sf.reshape(10, 128, 16)
    Wfdr = np.zeros((128, 8 * 32), np.float64)
    for a in range(8):
        for j in range(2):
            Wfdr[:, a * 32 + j * 16:a * 32 + j * 16 + 10] = \
                sfr[:, :, 2 * a + j].T
    bfv = bf.astype(np.float32).reshape(10, 1)

    return dict(
        A1=A1.astype(np.float16), th1=th1, th1n=-th1, th3n=-t3.astype(np.float32).reshape(128, 1),
        W2p=W2p.astype(bf16), b2p=b2p,
        W3Adr=W3Adr.astype(f8), W3Bdr=W3Bdr.astype(f8), th3=th3,
        W4dr=W4dr.astype(f8), b4p=b4p,
        Wfdr=Wfdr.astype(f8), bfv=bfv,
    )


def prepare_x(x_core):
    """x_core [B_CORE, 6, 128] fp32 -> (xh, xl) [128, B_CORE*6] bf16."""
    xT = np.ascontiguousarray(x_core.reshape(-1, 128).T.astype(np.float32))
    xh = xT.astype(np.float16)
    xl = (xT - xh.astype(np.float32)).astype(np.float16)
    return xh, xl


# ----------------------------------------------------------------------------
# Bass program (identical SPMD program per core)
# ----------------------------------------------------------------------------

def _ap(base, off, dims):
    """Raw strided AP view into a tile, dims = [(stride, n), ...] after the
    partition dim (which is copied from base)."""
    p = list(base.ap)[0]
    return AP(base.tensor, base.offset + off, [list(p)] + [list(d) for d in dims])


def build_program(n_chunks=B_CORE // NB):
    nc = bacc.Bacc("TRN2", target_bir_lowering=False, debug=False)
    bcore = n_chunks * NB

    xh_d = nc.dram_tensor("xh", [128, bcore * 6], F16, kind="ExternalInput").ap()
    xl_d = nc.dram_tensor("xl", [128, bcore * 6], F16, kind="ExternalInput").ap()
    A1_d = nc.dram_tensor("A1", [128, 32 * 128], F16, kind="ExternalInput").ap()
    th1_d = nc.dram_tensor("th1", [128, 1], F32, kind="ExternalInput").ap()
    th1n_d = nc.dram_tensor("th1n", [128, 1], F32, kind="ExternalInput").ap()
    th3n_d = nc.dram_tensor("th3n", [128, 1], F32, kind="ExternalInput").ap()
    W2_d = nc.dram_tensor("W2p", [128, 128], BF16, kind="ExternalInput").ap()
    b2_d = nc.dram_tensor("b2p", [128, 1], F32, kind="ExternalInput").ap()
    W3A_d = nc.dram_tensor("W3Adr", [128, 256], FP8, kind="ExternalInput").ap()
    W3B_d = nc.dram_tensor("W3Bdr", [128, 256], FP8, kind="ExternalInput").ap()
    th3_d = nc.dram_tensor("th3", [128, 1], F32, kind="ExternalInput").ap()
    W4_d = nc.dram_tensor("W4dr", [128, 3 * 256], FP8, kind="ExternalInput").ap()
    b4_d = nc.dram_tensor("b4p", [128, 1], F32, kind="ExternalInput").ap()
    Wf_d = nc.dram_tensor("Wfdr", [128, 8 * 32], FP8, kind="ExternalInput").ap()
    bf_d = nc.dram_tensor("bfv", [10, 1], F32, kind="ExternalInput").ap()

    y_d = nc.dram_tensor("y", [bcore, 10], F32, kind="ExternalOutput").ap()
    import os
    dbg = os.environ.get("KDEBUG") == "1"
    if dbg:
        dbg_s1 = nc.dram_tensor("dbg_s1", [128, BH * 16], F32, kind="ExternalOutput").ap()
        dbg_q = nc.dram_tensor("dbg_q", [128, BH * 18], F32, kind="ExternalOutput").ap()
        dbg_s3 = nc.dram_tensor("dbg_s3", [128, NB * 96], F32, kind="ExternalOutput").ap()
        dbg_s4 = nc.dram_tensor("dbg_s4", [128, 2 * NB * 16], F32, kind="ExternalOutput").ap()

    with tile.TileContext(nc) as tc:
        with (
            tc.tile_pool(name="consts", bufs=1) as consts,
            tc.tile_pool(name="xin", bufs=2) as xin_pool,
            tc.tile_pool(name="bt", bufs=4) as bt_pool,       # B' binarize tmps
            tc.tile_pool(name="oc", bufs=2) as oc_pool,
            tc.tile_pool(name="pA", bufs=5, space="PSUM") as pA,   # L1+L3 384
            tc.tile_pool(name="pB", bufs=2, space="PSUM") as pB,   # L2+L4 512
            tc.tile_pool(name="pF", bufs=1, space="PSUM") as pF,   # fc
        ):
            # --- constants ---
            A1_s = consts.tile([128, 32 * 128], F16)
            nc.sync.dma_start(out=A1_s, in_=A1_d)
            th1_s = consts.tile([128, 1], F32)
            nc.sync.dma_start(out=th1_s, in_=th1_d)
            th1n_s = consts.tile([128, 1], F32)
            nc.sync.dma_start(out=th1n_s, in_=th1n_d)
            th3n_s = consts.tile([128, 1], F32)
            nc.sync.dma_start(out=th3n_s, in_=th3n_d)
            W2_s = consts.tile([128, 128], BF16)
            nc.sync.dma_start(out=W2_s, in_=W2_d)
            b2_s = consts.tile([128, 1], F32)
            nc.sync.dma_start(out=b2_s, in_=b2_d)
            W3A_s = consts.tile([128, 256], FP8)
            nc.sync.dma_start(out=W3A_s, in_=W3A_d)
            W3B_s = consts.tile([128, 256], FP8)
            nc.sync.dma_start(out=W3B_s, in_=W3B_d)
            th3_s = consts.tile([128, 1], F32)
            nc.sync.dma_start(out=th3_s, in_=th3_d)
            W4_s = consts.tile([128, 3 * 256], FP8)
            nc.sync.dma_start(out=W4_s, in_=W4_d)
            b4_s = consts.tile([128, 1], F32)
            nc.sync.dma_start(out=b4_s, in_=b4_d)
            Wf_s = consts.tile([128, 8 * 32], FP8)
            nc.sync.dma_start(out=Wf_s, in_=Wf_d)
            bf_s = consts.tile([10, 1], F32)
            nc.sync.dma_start(out=bf_s, in_=bf_d)

            W3A_v = W3A_s.rearrange("p (j m) -> p j m", j=2)
            W3B_v = W3B_s.rearrange("p (j m) -> p j m", j=2)
            W4_v = W4_s.rearrange("p (a j m) -> p a j m", a=3, j=2)
            Wf_v = Wf_s.rearrange("p (a j m) -> p a j m", a=8, j=2, m=16)

            # --- persistent double-buffered activations ---
            s1t = [consts.tile([128, BH * 16], BF16, name=f"s1t{i}") for i in range(2)]
            qt = [consts.tile([128, BH * 18], FP8, name=f"qt{i}") for i in range(2)]
            s3t = [consts.tile([128, NB * 96], FP8, name=f"s3t{i}") for i in range(2)]  # (b,u,h)
            s4t = [consts.tile([128, 2 * NB * 16], FP8, name=f"s4t{i}") for i in range(2)]

            # one-time pads: s1 pad cells = 0.5 (so 2*s-1 = 0), q pads = 0
            for t in s1t:
                nc.gpsimd.memset(t[0:32, 0:384], 0.5)
                nc.gpsimd.memset(t[96:128, 15 * 384:16 * 384], 0.5)
            for t in qt:
                nc.gpsimd.memset(t[:, 0:BH], 0.0)
                nc.gpsimd.memset(t[:, 17 * BH:18 * BH], 0.0)
            if dbg:
                for t in s4t:
                    nc.gpsimd.memset(t, 0.0)

            eng = [nc.vector, nc.gpsimd]

            for it in range(n_chunks + 1):
                # ================= L1 for chunk c = it =================
                if it < n_chunks:
                    c = it
                    xh_s = xin_pool.tile([128, BH], F16, tag="xh")
                    nc.sync.dma_start(out=xh_s, in_=xh_d[:, c * BH:(c + 1) * BH])
                    xl_s = xin_pool.tile([128, BH], F16, tag="xl")
                    nc.sync.dma_start(out=xl_s, in_=xl_d[:, c * BH:(c + 1) * BH])
                    s1r = s1t[c % 2]
                    for m in range(16):
                        psa = pA.tile([128, BH], F32, tag="pp")
                        psb = pA.tile([128, BH], F32, tag="pp")
                        sa = slice((2 * m) * 128, (2 * m + 1) * 128)
                        sb = slice((2 * m + 1) * 128, (2 * m + 2) * 128)
                        nc.tensor.matmul(psa, A1_s[:, sa], xh_s, start=True, stop=False)
                        nc.tensor.matmul(psa, A1_s[:, sa], xl_s, start=False, stop=True)
                        nc.tensor.matmul(psb, A1_s[:, sb], xh_s, start=True, stop=False)
                        nc.tensor.matmul(psb, A1_s[:, sb], xl_s, start=False, stop=True)
                        b1p = bt_pool.tile([128, BH], BF16, tag="b1")
                        if m % 4 != 3:
                            # ACT: +-1; max({0,1}, +-1) still yields {0,1}
                            nc.scalar.activation(b1p, psb, AF.Sign, bias=th1n_s)
                        else:
                            nc.vector.tensor_scalar(out=b1p, in0=psb, scalar1=th1_s,
                                                    scalar2=None, op0=ALU.is_ge)
                        if m == 0:
                            slices = [slice(32, 64), slice(64, 128)]
                        elif m == 15:
                            slices = [slice(0, 96)]
                        else:
                            slices = [slice(0, 128)]
                        for rs in slices:
                            nc.vector.scalar_tensor_tensor(
                                out=s1r[rs, 384 * m:384 * (m + 1)], in0=psa[rs],
                                scalar=th1_s[rs],
                                in1=b1p[rs], op0=ALU.is_ge, op1=ALU.max)

                # ============ L2/L3/L4/fc for chunk c = it-1 ============
                if it >= 1:
                    c = it - 1
                    s1c = s1t[c % 2]
                    qb = qt[c % 2]
                    s3c = s3t[c % 2]
                    # ---- L2: 12 blocks of (32 bh x 16 u) ----
                    for s in range(12):
                        ps2 = pB.tile([128, 512], F32, tag="pb")
                        nc.tensor.matmul(ps2, W2_s,
                                         s1c[:, 512 * s:512 * (s + 1)],
                                         start=True, stop=True)
                        nc.scalar.activation(
                            qb[:, BH + 512 * s:BH + 512 * (s + 1)],
                            ps2, AF.Sign, bias=b2_s)
                    # ---- L3: DoubleRow, 12 raster blocks of 512 per parity ----
                    for k in range(12):
                        p3a = pA.tile([128, 512], F32, tag="pp")
                        p3b = pA.tile([128, 512], F32, tag="pp")
                        rha = _ap(qb, 512 * k, [(BH, 2), (1, 512)])
                        rhb = _ap(qb, 512 * k + BH, [(BH, 2), (1, 512)])
                        nc.tensor.matmul(p3a, W3A_v, rha, start=True, stop=True,
                                         perf_mode=DR)
                        nc.tensor.matmul(p3b, W3B_v, rhb, start=True, stop=True,
                                         perf_mode=DR)
                        b3p = bt_pool.tile([128, 512], BF16, tag="b3")
                        if k % 6 != 5:
                            nc.scalar.activation(b3p, p3b, AF.Sign, bias=th3n_s)
                        else:
                            nc.vector.tensor_scalar(out=b3p, in0=p3b, scalar1=th3_s,
                                                    scalar2=None, op0=ALU.is_ge)
                        nc.vector.scalar_tensor_tensor(
                            out=s3c[:, 512 * k:512 * (k + 1)], in0=p3a,
                            scalar=th3_s, in1=b3p, op0=ALU.is_ge, op1=ALU.max)
                    # ---- L4: DoubleRow over h-pairs, 2 b-halves ----
                    ps4 = [pB.tile([128, 512], F32, tag="pb", name=f"ps4_{half}") for half in range(2)]
                    for a in range(3):
                        for half in range(2):
                            rh4 = _ap(s3c, half * 3072 + 2 * a,
                                      [(1, 2), (6, 512)])
                            nc.tensor.matmul(ps4[half], W4_v[:, a], rh4,
                                             start=(a == 0), stop=(a == 2),
                                             perf_mode=DR)
                    s4g = s4t[(c // 2) % 2]
                    cc = c % 2
                    for half in range(2):
                        nc.scalar.activation(
                            s4g[:, cc * 1024 + half * 512:cc * 1024 + (half + 1) * 512],
                            ps4[half], AF.Sign, bias=b4_s)
                    # ---- fc: DoubleRow over w-pairs, once per 2 chunks ----
                    if c % 2 == 1 or c == n_chunks - 1:
                        ncc = (c % 2) + 1            # chunks in this group
                        nn = ncc * NB
                        psf = pF.tile([16, 2 * NB], F32, tag="pf")
                        for cc2 in range(ncc):
                            for a in range(8):
                                rhf = _ap(s4g, cc2 * 1024 + 128 * a,
                                          [(64, 2), (1, 64)])
                                nc.tensor.matmul(psf[:, cc2 * 64:(cc2 + 1) * 64],
                                                 Wf_v[:, a], rhf,
                                                 start=(a == 0), stop=(a == 7),
                                                 perf_mode=DR, skip_group_check=True)
                        outc = oc_pool.tile([10, 2 * NB], F32)
                        nc.vector.tensor_scalar_add(outc[:, 0:nn], psf[0:10, 0:nn], bf_s)
                        g0 = (c // 2) * 2 * NB
                        nc.sync.dma_start(
                            out=y_d[g0:g0 + nn, :].rearrange("b j -> j b"),
                            in_=outc[:, 0:nn])

            if dbg:
                d1 = consts.tile([128, BH * 16], F32)
                nc.vector.tensor_copy(d1, s1t[0])
                nc.sync.dma_start(out=dbg_s1, in_=d1)
                d2 = consts.tile([128, BH * 18], F32)
                nc.vector.tensor_copy(d2, qt[0])
                nc.sync.dma_start(out=dbg_q, in_=d2)
                d3 = consts.tile([128, NB * 96], F32)
                nc.vector.tensor_copy(d3, s3t[0])
                nc.sync.dma_start(out=dbg_s3, in_=d3)
                d4 = consts.tile([128, 2 * NB * 16], F32)
                nc.vector.tensor_copy(d4, s4t[0])
                nc.sync.dma_start(out=dbg_s4, in_=d4)

    nc.compile()
    return nc


_PROGRAM = None


def _get_program():
    global _PROGRAM
    if _PROGRAM is None:
        _PROGRAM = build_program()
    return _PROGRAM


def run(trace=False, **inputs):
    inputs = {k: np.asarray(v) for k, v in inputs.items()}
    consts = prepare_host_tensors(
        **{k: inputs[k] for k in
           ("w1", "b1", "w2", "b2", "w3", "b3", "w4", "b4",
            "g1", "be1", "m1", "v1", "g2", "be2", "m2", "v2",
            "g3", "be3", "m3", "v3", "g4", "be4", "m4", "v4", "wf", "bf")})
    x = inputs["x"].astype(np.float32)           # [8192, 1, 6, 128]
    nc = _get_program()
    in_maps = []
    for k in range(N_CORES):
        xh, xl = prepare_x(x[k * B_CORE:(k + 1) * B_CORE, 0])
        m = {"xh": xh, "xl": xl}
        m.update(consts)
        in_maps.append(m)
    res = run_bass_kernel_spmd(nc, in_maps, list(range(N_CORES)), trace=trace)
    y = np.concatenate([r["y"] for r in res.results], axis=0)
    return y.astype(np.float32), res


def kernel(**inputs):
    y, _ = run(trace=False, **inputs)
    return y
